# revision 84
# baseline (speedup 1.0000x reference)
"""Trainium2 Bass kernel v6 for nn_Attention_18760417149505.

Reference (per problem):
  q/k/v = (x @ W.T + b).reshape(B, H, S, dk)      # flat reshape, NOT head-split
  scores = q @ k.T ; t = (scores*SCALE) @ v ; attn = softmax(t, axis=-1)
  out = ((attn.reshape(B,S,D) @ Wo.T + bo) @ Wf.T + bf)

Softmax comes AFTER both score matmuls, so per (batch, head) only the 64x64
Gram matrix G = SCALE * k.T @ v is needed:  t = q @ G.

v10 design (fp16 matmul path, folded Wf@Wo, per-core-local heads as in v2);
cost-model profile 52.5us vs the v2 phase-split baseline's 57.3us:
  * MERGED schedule: no phase barrier.  K/V/Q projections emit
    256-column ranges so head 0's carve/G/T/softmax pipeline starts
    ~14us in and later heads' chains hide under the remaining
    projection + output matmuls (the Tile scheduler is dependency-
    driven; emission order sets priorities and the column-range
    emission shapes the dep graph to allow the overlap).
  * T runs in NINE (head, J-pair third) batches of 8 chunks: the T psum
    tile is [128,8,64]f32 = ONE bank (vs 2 for the v2 12-chunk batch),
    which frees enough banks for BOTH the carve pool and the attention-
    transpose pool to get two alternating banks (no group-WAR
    serialization).  G accumulates in tiles borrowed from the T pool.
  * Softmax per batch: DVE negmax from the T psum, DVE broadcast-add
    writes the shifted logits to an fp16 SBUF pre-tile (the PE selector
    shift of v2..v7 is gone: the front of the kernel is PE-bound, so
    moving the shift to DVE bought ~2us even though DVE pays 925ns),
    ACT exp from SBUF, DVE sum+recip, Pool normalize, PE transposes +
    DVE/ACT copies into feature-major mt.  Output-range drains also run
    on DVE (bias via broadcast-AP tensor_add): the back half has DVE
    slack while ACT carries the exps.
  * DMAs: SP/ACT (HW DGE) + Pool (SW DGE); engine-occupancy equals
    transfer time, so ACT carries ONLY the x[:, :256] input transfer --
    every outT DMA issue on ACT head-of-line blocked an exp in the
    chain stream (moving them all to SP/Pool bought 1.2us).  Inputs are
    column-merged (one DMA per consumer-aligned column split: x halves,
    weights j<2 | j>=2) cutting 26 transfers to 10; first-needed
    columns (wk j0-1, x cols 0:256) land ~3us in; bias rides SP right
    after wk_a (queue priority = emission position).
  * PSUM: proj/out 2x[128,512]f32, carve 2x[128,4,2,128]f16,
    atr/nmT 2x[128,4,128]f16, T/G 2x[128,8,64]f32 = 8 banks exactly.
  * Q-half2 emits per J-pair interleaved with the T batches of head 1
    (each T third needs only its own J-pair), pulling the back-half
    chain stream ~2us earlier; the last output range pre-accumulates
    k0..3 of two blocks in borrowed T-pool psum while the final chain
    runs, and finishes the short blocks last so the final DMA issues
    early.
  * NOTE: XBAR dma_start_transpose matches CoreSim but produces wrong
    results through the compiled NEFF path (tried in v4) -- all
    transposes stay on the PE.

Sharding: flat reshape makes head h own flat rows [2048h, 2048(h+1)) of the
[B*24576, 64] flat view == rows [512c, 512(c+1)) of the [4096, 768] (B*S, D)
matrix for head-triple c. Core c gets x rows [512c, 512(c+1)) and heads
{3c, 3c+1, 3c+2} - fully local, no collectives.
"""

import numpy as np

import concourse.bass as bass
import concourse.mybir as mybir
import concourse.tile as tile
from concourse import bacc
from concourse.bass_utils import run_bass_kernel_spmd
from concourse.masks import make_identity

F32 = mybir.dt.float32
F16 = mybir.dt.float16

B, S, D = 2, 2048, 768
H, DK = 12, 64
SCALE = 0.125
NCORES = 8
SLOC = 512          # x rows per core
HLOC = 3            # heads per core
NCH = 24            # carve chunks per head (12 groups x 2)
KT_COLS = 640       # kt/vt padded so c=1 carve transposes are full [128,128]
QT_COLS = 640       # qt tiles padded so c=1 T-chunks never clamp
MT_COLS = 600       # mt tiles padded for the 128-wide transposed writes

ACT_ID = mybir.ActivationFunctionType.Identity
ACT_EXP = mybir.ActivationFunctionType.Exp


def _ceil_div(a, b):
    return -((-a) // b)


def _slabs():
    """Per (head l, group g): local x-row range [s_lo, s_hi) of the slab."""
    tab = {}
    for l in range(HLOC):
        tot = 0
        for g in range(12):
            s_lo = max(0, _ceil_div(2048 * l - g, 12))
            s_hi = min(SLOC, _ceil_div(2048 * (l + 1) - g, 12))
            tab[(l, g)] = (s_lo, s_hi)
            tot += s_hi - s_lo
        assert tot == 2048, tot
    for l in range(HLOC):
        for j in range(6):
            assert tab[(l, 2 * j)] == tab[(l, 2 * j + 1)]
    return tab


SLABS = _slabs()

# output projection x-row ranges (head regions; boundary cols go with the
# later head so each range only needs heads <= its index complete)
O_RANGES = [(0, 170), (170, 341), (341, SLOC)]


def build_nc():
    nc = bacc.Bacc()

    xT = nc.declare_dram_parameter("xT", [D, SLOC], F16, isOutput=False)
    wqT = nc.declare_dram_parameter("wqT", [D, D], F16, isOutput=False)
    wkT = nc.declare_dram_parameter("wkT", [D, D], F16, isOutput=False)
    wvT = nc.declare_dram_parameter("wvT", [D, D], F16, isOutput=False)
    wfoT = nc.declare_dram_parameter("wfoT", [D, D], F16, isOutput=False)
    # [:, i, j] = b_i[128j+p] for i in (q, k, v, fo') with bfo' = Wf@bo+bf
    bias_po = nc.declare_dram_parameter("bias_po", [128, 4, 6], F32, isOutput=False)
    outT = nc.declare_dram_parameter("outT", [D, SLOC], F16, isOutput=True)

    with tile.TileContext(nc) as tc:
        with (
            tc.tile_pool(name="consts", bufs=1) as consts,
            tc.tile_pool(name="xw", bufs=1) as xwp,
            tc.tile_pool(name="ktv", bufs=1) as ktvp,
            tc.tile_pool(name="kvc", bufs=1) as kvcp,
            tc.tile_pool(name="qt", bufs=1) as qtp,
            tc.tile_pool(name="gsb", bufs=1) as gsbp,
            tc.tile_pool(name="mt", bufs=1) as mtp,
            tc.tile_pool(name="sm", bufs=4) as smp,
            tc.tile_pool(name="smx", bufs=4) as smxp,
            tc.tile_pool(name="outp", bufs=1) as outp,
            tc.tile_pool(name="pp", bufs=2, space="PSUM") as pp,
            tc.tile_pool(name="ptr", bufs=2, space="PSUM") as ptrp,
            tc.tile_pool(name="ptq", bufs=1, space="PSUM") as ptqp,
            tc.tile_pool(name="ppt", bufs=2, space="PSUM") as ppt,
        ):
            # ---- constants ------------------------------------------------
            ident = consts.tile([128, 128], F16)
            make_identity(nc, ident)
            bias_sb = consts.tile([128, 4, 6], F32)
            # pay the Exp act-table load during the DMA phase, not mid-softmax
            warm = consts.tile([1, 1], F32)
            nc.vector.memset(warm, 0.0)
            nc.scalar.activation(warm, warm, ACT_EXP)

            # ---- input DMAs ----------------------------------------------
            # Column-merged transfers: each tensor as an (a, b) column split
            # so the first-needed columns land in one early transfer.  x
            # splits by x-column half (proj c-ranges); weights split by
            # output-feature block (j<2 | j>=2).
            x_a = xwp.tile([128, 6, 256], F16)
            x_b = xwp.tile([128, 6, 256], F16)
            wk_a = xwp.tile([128, 6, 256], F16)
            wk_b = xwp.tile([128, 6, 512], F16)
            wv_a = xwp.tile([128, 6, 256], F16)
            wv_b = xwp.tile([128, 6, 512], F16)
            wq_a = xwp.tile([128, 6, 256], F16)
            wq_b = xwp.tile([128, 6, 512], F16)
            wfo_a = xwp.tile([128, 6, 256], F16)
            wfo_b = xwp.tile([128, 6, 512], F16)

            def xT_ap(k, c0, c1):
                if c1 <= 256:
                    return x_a[:, k, c0:c1]
                return x_b[:, k, c0 - 256:c1 - 256]

            def _wblk(a, b):
                def get(k, j):
                    if j < 2:
                        return a[:, k, 128 * j:128 * (j + 1)]
                    return b[:, k, 128 * (j - 2):128 * (j - 1)]
                return get

            wkT_sb = _wblk(wk_a, wk_b)
            wvT_sb = _wblk(wv_a, wv_b)
            wqT_sb = _wblk(wq_a, wq_b)
            wfoT_sb = _wblk(wfo_a, wfo_b)

            def cdma(eng, t, dram, c0, c1, rowlen=D):
                full = dram[:, :]
                ap = bass.AP(tensor=full.tensor, offset=c0,
                             ap=[[rowlen, 128], [128 * rowlen, 6],
                                 [1, c1 - c0]])
                eng.dma_start(out=t, in_=ap)

            # SP: wk_a, x_b, wv_a, wfo, sel
            cdma(nc.sync, wk_a, wkT, 0, 256)
            nc.sync.dma_start(out=bias_sb, in_=bias_po[:, :, :])
            cdma(nc.sync, x_b, xT, 256, 512, rowlen=SLOC)
            cdma(nc.sync, wv_a, wvT, 0, 256)
            cdma(nc.sync, wfo_a, wfoT, 0, 256)
            cdma(nc.sync, wfo_b, wfoT, 256, 768)
            # ACT: x_a only (free from ~1.5us for proj drains)
            cdma(nc.scalar, x_a, xT, 0, 256, rowlen=SLOC)
            # Pool: wk_b, wv_b, wq
            cdma(nc.gpsimd, wk_b, wkT, 256, 768)
            cdma(nc.gpsimd, wv_b, wvT, 256, 768)
            cdma(nc.gpsimd, wq_a, wqT, 0, 256)
            cdma(nc.gpsimd, wq_b, wqT, 256, 768)

            # ---- persistent tiles ----------------------------------------
            kt_sb = [ktvp.tile([128, KT_COLS], F16, tag=f"kt{j}", name=f"kt{j}")
                     for j in range(6)]
            vt_sb = [ktvp.tile([128, KT_COLS], F16, tag=f"vt{j}", name=f"vt{j}")
                     for j in range(6)]
            # packed per-head carve: k at [:, 0, ch, :], v at [:, 1, ch, :]
            # chunk index ch = 4j + 2c + h2
            kvc = [kvcp.tile([128, 2, NCH, DK], F16, tag=f"kvc{l}",
                             name=f"kvc{l}") for l in range(HLOC)]
            qt_sb = [qtp.tile([128, QT_COLS], F16, tag=f"qt{j}", name=f"qt{j}")
                     for j in range(6)]
            # zero-padded G parity variants: [:, 0, l, :] = [G_l; 0],
            # [:, 1, l, :] = [0; G_l]
            g_sb = gsbp.tile([128, 2, HLOC, DK], F16)
            nc.scalar.memzero(g_sb[:, :, :, :])
            mt_sb = [mtp.tile([128, MT_COLS], F16, tag=f"mt{j}", name=f"mt{j}")
                     for j in range(6)]
            out_sb = [outp.tile([128, SLOC], F16, tag=f"ou{j}", name=f"ou{j}")
                      for j in range(6)]
            # zero padding cols (read by unclamped c=1 chunks); only
            # kt/vt pads are needed early (carve(2) ~10us); qt pads by T(2,*)
            for j in range(6):
                nc.vector.memset(kt_sb[j][:, SLOC:KT_COLS], 0.0)
                nc.vector.memset(vt_sb[j][:, SLOC:KT_COLS], 0.0)
            for j in range(6):
                nc.vector.memset(qt_sb[j][:, SLOC:QT_COLS], 0.0)

            # carve transpose psum: two alternating banks
            trp = [ptrp.tile([128, 4, 2, 128], F16, tag="trp",
                             name=f"trp{i}") for i in range(2)]
            # atr + nmT transpose psum: two alternating banks
            trqs = [ptqp.tile([128, 4, 128], F16, tag=f"trq{i}",
                              name=f"trq{i}") for i in range(2)]
            trq_n = [0]

            # ---- emission helpers ----------------------------------------
            def proj_cols(bi, w_sb, dst, j, c0, c1, dve_drain=False):
                """Feature-major projection block j over x-col range [c0,c1)."""
                ln = c1 - c0
                ps = pp.tile([128, 512], F32, tag="pp")
                for k in range(6):
                    nc.tensor.matmul(
                        ps[:, 0:ln],
                        w_sb(k, j),
                        xT_ap(k, c0, c1),
                        start=(k == 0), stop=(k == 5),
                    )
                if dve_drain:
                    b = bias_sb[:, bi, j:j + 1]
                    b_b = bass.AP(tensor=b.tensor, offset=b.offset,
                                  ap=[b.ap[0], [0, ln]])
                    nc.vector.tensor_add(dst[:, c0:c1], ps[:, 0:ln], b_b)
                else:
                    nc.scalar.activation(
                        dst[:, c0:c1], ps[:, 0:ln], ACT_ID,
                        bias=bias_sb[:, bi, j:j + 1],
                    )

            cdma_n = [0]

            def carve(l):
                """Per-head packed [rows, dk] k/v chunks: PE transposes in
                4-slot groups through two alternating banks, one wide copy
                per group."""
                for jp in range(3):
                    tr = trp[(3 * l + jp) % 2]
                    for dj in range(2):
                        j = 2 * jp + dj
                        s_lo, _ = SLABS[(l, 2 * j)]
                        for c in range(2):
                            s0 = s_lo + 128 * c
                            slot = 2 * dj + c
                            nc.tensor.transpose(
                                tr[:, slot, 0, :],
                                kt_sb[j][:, s0:s0 + 128], ident)
                            nc.tensor.transpose(
                                tr[:, slot, 1, :],
                                vt_sb[j][:, s0:s0 + 128], ident)
                    srcap = bass.AP(
                        tensor=tr.tensor, offset=tr.offset,
                        ap=[tr.ap[0], [128, 2], [256, 4], [64, 2], [1, 64]])
                    kt = kvc[l]
                    dstap = bass.AP(
                        tensor=kt.tensor,
                        offset=kt.offset + 8 * jp * DK,
                        ap=[kt.ap[0], [NCH * DK, 2], [2 * DK, 4],
                            [DK, 2], [1, DK]])
                    if jp % 2 == 0:
                        nc.vector.tensor_copy(dstap, srcap)
                    else:
                        nc.scalar.copy(dstap, srcap)

            def g_head(l):
                gps = ppt.tile([DK, HLOC, DK], F32, tag="T", name=f"g{l}")
                pieces = []
                for g in range(12):
                    s_lo, s_hi = SLABS[(l, g)]
                    L = s_hi - s_lo
                    j, h2 = g // 2, g % 2
                    pieces.append((4 * j + h2, min(128, L)))
                    if L > 128:
                        pieces.append((4 * j + 2 + h2, L - 128))
                for i, (ch, kk) in enumerate(pieces):
                    nc.tensor.matmul(
                        gps[:, l, :],
                        kvc[l][0:kk, 0, ch, :],
                        kvc[l][0:kk, 1, ch, :],
                        start=(i == 0), stop=(i == len(pieces) - 1),
                    )
                nc.vector.tensor_scalar_mul(
                    g_sb[0:DK, 0, l, :], gps[:, l, :], SCALE)
                nc.vector.tensor_copy(
                    g_sb[DK:128, 1, l, :], g_sb[0:DK, 0, l, :])

            # per-batch softmax state
            tps_t = {}
            sm_t = {}
            nm_t = {}

            def t_mms(l, t):
                """T = q @ G chunks for batch (l, third t) -> psum
                [128, 8, 64] (one bank, one accumulation group)."""
                tps = ppt.tile([128, 8, DK], F32, tag="T", name=f"T{l}{t}")
                tps_t[(l, t)] = tps
                for jj in range(2):
                    J = 2 * t + jj
                    for c in range(2):
                        for p in range(2):
                            g = 4 * t + 2 * jj + p
                            s_lo, _ = SLABS[(l, g)]
                            col0 = s_lo + 128 * c
                            ch = 4 * jj + 2 * c + p
                            nc.tensor.matmul(
                                tps[:, ch, :],
                                qt_sb[J][:, col0:col0 + 128],
                                g_sb[:, p, l, :],
                                start=(ch == 0), stop=(ch == 7),
                            )

            def sm_a(l, hf):
                """negmax of the T psum (DVE)."""
                tps = tps_t[(l, hf)]
                negmax = smxp.tile([128, 8], F16, tag="nm", name=f"nm{l}{hf}")
                with nc.allow_low_precision(reason="shift only needs ~ulp(max)"):
                    nc.vector.reduce_max(negmax, tps, axis=mybir.AxisListType.X,
                                         negate=True)
                nm_t[(l, hf)] = negmax

            def sm_add(l, hf):
                """DVE broadcast-add shift: psum -> fp16 SBUF pre-tile.
                Used for front batches where DVE is idle and PE is the
                constraint (saves the PE transpose + selector matmul)."""
                tps = tps_t[(l, hf)]
                negmax = nm_t[(l, hf)]
                pre = smp.tile([128, 8, DK], F16, tag="P", name=f"P{l}{hf}")
                nm_b = bass.AP(tensor=negmax.tensor, offset=negmax.offset,
                               ap=[negmax.ap[0], negmax.ap[1], [0, DK]])
                nc.vector.tensor_add(pre, tps, nm_b)
                tps_t[(l, hf)] = pre

            def sm_b(l, hf):
                """exp (ACT), sum+recip (DVE), normalize (Pool/GPSIMD)."""
                tps = tps_t[(l, hf)]
                av = smp.tile([128, 8, DK], F16, tag="A", name=f"A{l}{hf}")
                sm_t[(l, hf)] = av
                nc.scalar.activation(av, tps, ACT_EXP)
                s = smxp.tile([128, 8], F16, tag="sm", name=f"sm{l}{hf}")
                inv = smxp.tile([128, 8], F16, tag="inv", name=f"inv{l}{hf}")
                with nc.allow_low_precision(reason="exp sums in [1,64], fp16 ok"):
                    nc.vector.reduce_sum(s, av, axis=mybir.AxisListType.X)
                    nc.vector.reciprocal(inv, s)
                inv_b = bass.AP(tensor=inv.tensor, offset=inv.offset,
                                ap=[inv.ap[0], inv.ap[1], [0, DK]])
                nc.gpsimd.tensor_mul(av, av, inv_b)

            def atr(l, t):
                """PE-transpose attention batch (l, third t) into mt."""
                av = sm_t[(l, t)]
                for jj in range(2):
                    J = 2 * t + jj
                    g = 4 * t + 2 * jj
                    s_lo, _ = SLABS[(l, g)]
                    trq = trqs[trq_n[0] % 2]
                    trq_n[0] += 1
                    for c in range(2):
                        ch = 4 * jj + 2 * c
                        nc.tensor.transpose(
                            trq[:, c, :], av[:, ch:ch + 2, :], ident)
                    srcap = bass.AP(
                        tensor=trq.tensor, offset=trq.offset,
                        ap=[trq.ap[0], [1, 256]])
                    if (2 * t + jj) % 2 == 0:
                        nc.vector.tensor_copy(
                            mt_sb[J][:, s_lo:s_lo + 256], srcap)
                    else:
                        nc.scalar.copy(
                            mt_sb[J][:, s_lo:s_lo + 256], srcap)

            odma_n = [0]

            def o_range(r):
                r0, r1 = O_RANGES[r]
                ln = r1 - r0
                for j in range(6):
                    ps = pp.tile([128, 512], F32, tag="pp")
                    for k in range(6):
                        nc.tensor.matmul(
                            ps[:, 0:ln],
                            wfoT_sb(k, j),
                            mt_sb[k][:, r0:r1],
                            start=(k == 0), stop=(k == 5),
                        )
                    nc.scalar.activation(
                        out_sb[j][:, r0:r1], ps[:, 0:ln], ACT_ID,
                        bias=bias_sb[:, 3, j:j + 1],
                    )
                    eng = [nc.sync, nc.gpsimd][odma_n[0] % 2]
                    odma_n[0] += 1
                    eng.dma_start(
                        out=outT[128 * j:128 * (j + 1), r0:r1],
                        in_=out_sb[j][:, r0:r1])

            def chain(l, hf):
                sm_a(l, hf)
                sm_add(l, hf)
                sm_b(l, hf)

            def o2_pre():
                """Pre-accumulate k0..3 for the first output blocks of the
                last range in borrowed T-pool psum (open groups), while the
                final softmax chain still runs."""
                r0, r1 = O_RANGES[2]
                ln = r1 - r0
                pre = {}
                for j in (0, 1):
                    ps = ppt.tile([128, 8, DK], F32, tag="T", name=f"o2p{j}")
                    psf = bass.AP(tensor=ps.tensor, offset=ps.offset,
                                  ap=[ps.ap[0], [1, ln]])
                    for k in range(4):
                        nc.tensor.matmul(psf, wfoT_sb(k, j),
                                         mt_sb[k][:, r0:r1],
                                         start=(k == 0), stop=False)
                    pre[j] = psf
                return pre

            def o2_finish(pre):
                r0, r1 = O_RANGES[2]
                ln = r1 - r0
                for j in (2, 3, 4, 5, 0, 1):
                    if j in pre:
                        psf = pre[j]
                        for k in (4, 5):
                            nc.tensor.matmul(psf, wfoT_sb(k, j),
                                             mt_sb[k][:, r0:r1],
                                             start=False, stop=(k == 5))
                    else:
                        ps = pp.tile([128, 512], F32, tag="pp")
                        psf = ps[:, 0:ln]
                        for k in range(6):
                            nc.tensor.matmul(psf, wfoT_sb(k, j),
                                             mt_sb[k][:, r0:r1],
                                             start=(k == 0), stop=(k == 5))
                    b = bias_sb[:, 3, j:j + 1]
                    b_b = bass.AP(tensor=b.tensor, offset=b.offset,
                                  ap=[b.ap[0], [0, ln]])
                    nc.vector.tensor_add(out_sb[j][:, r0:r1], psf, b_b)
                    eng = [nc.sync, nc.gpsimd][odma_n[0] % 2]
                    odma_n[0] += 1
                    eng.dma_start(
                        out=outT[128 * j:128 * (j + 1), r0:r1],
                        in_=out_sb[j][:, r0:r1])

            # ---- merged emission schedule --------------------------------
            for j in range(6):
                proj_cols(1, wkT_sb, kt_sb[j], j, 0, 256)
            for j in range(6):
                proj_cols(2, wvT_sb, vt_sb[j], j, 0, 256)
            carve(0)
            for j in range(6):
                proj_cols(0, wqT_sb, qt_sb[j], j, 0, 256)
            g_head(0)
            t_mms(0, 0); chain(0, 0)
            for j in range(6):
                proj_cols(1, wkT_sb, kt_sb[j], j, 256, SLOC)
            t_mms(0, 1); chain(0, 1)
            for j in range(6):
                proj_cols(2, wvT_sb, vt_sb[j], j, 256, SLOC)
            t_mms(0, 2); chain(0, 2)
            atr(0, 0); atr(0, 1); atr(0, 2)
            carve(1); carve(2)
            g_head(1); g_head(2)
            proj_cols(0, wqT_sb, qt_sb[0], 0, 256, SLOC)
            proj_cols(0, wqT_sb, qt_sb[1], 1, 256, SLOC)
            t_mms(1, 0); chain(1, 0)
            proj_cols(0, wqT_sb, qt_sb[2], 2, 256, SLOC)
            proj_cols(0, wqT_sb, qt_sb[3], 3, 256, SLOC)
            t_mms(1, 1); chain(1, 1)
            proj_cols(0, wqT_sb, qt_sb[4], 4, 256, SLOC)
            proj_cols(0, wqT_sb, qt_sb[5], 5, 256, SLOC)
            o_range(0)
            t_mms(1, 2); chain(1, 2)
            t_mms(2, 0); chain(2, 0)
            atr(1, 0); atr(1, 1); atr(1, 2)
            t_mms(2, 1); chain(2, 1)
            o_range(1)
            t_mms(2, 2); chain(2, 2)
            atr(2, 0); atr(2, 1)
            pre = o2_pre()
            atr(2, 2)
            o2_finish(pre)

    nc.finalize()
    return nc


_NC_CACHE = None


def make_in_maps(x, Wq, bq, Wk, bk, Wv, bv, Wo, bo, Wf, bf):
    f32 = np.float32
    xf = np.asarray(x, f32).reshape(B * S, D)
    Wfo = np.asarray(Wf, f32) @ np.asarray(Wo, f32)
    bfo = np.asarray(Wf, f32) @ np.asarray(bo, f32) + np.asarray(bf, f32)
    shared = {
        "wqT": np.ascontiguousarray(np.asarray(Wq, f32).T).astype(np.float16),
        "wkT": np.ascontiguousarray(np.asarray(Wk, f32).T).astype(np.float16),
        "wvT": np.ascontiguousarray(np.asarray(Wv, f32).T).astype(np.float16),
        "wfoT": np.ascontiguousarray(Wfo.T).astype(np.float16),
        "bias_po": np.stack(
            [np.asarray(b, f32).reshape(6, 128).T
             for b in (bq, bk, bv, bfo)],
            axis=1,
        ).copy(),
    }
    in_maps = []
    for c in range(NCORES):
        m = dict(shared)
        m["xT"] = np.ascontiguousarray(
            xf[SLOC * c:SLOC * (c + 1), :].T).astype(np.float16)
        in_maps.append(m)
    return in_maps


def kernel(**inputs):
    global _NC_CACHE
    if _NC_CACHE is None:
        _NC_CACHE = build_nc()
    nc = _NC_CACHE
    in_maps = make_in_maps(**inputs)
    res = run_bass_kernel_spmd(nc, in_maps, list(range(NCORES)))
    out = np.empty((B * S, D), np.float32)
    for c in range(NCORES):
        out[SLOC * c:SLOC * (c + 1), :] = res.results[c]["outT"].T.astype(np.float32)
    return out.reshape(B, S, D)
